# revision 20
# baseline (speedup 1.0000x reference)
"""Multi-head causal attention (B=4, T=1024, C=1024, H=16, D=64) on 8 TRN2 cores.

Sharding: tensor-parallel over heads. Core i owns heads {2i, 2i+1}:
  - x is replicated (sent pre-transposed as xT [C, B*T], bf16)
  - Wq/Wk/Wv sharded over heads -> per-core [C, 128] (2 heads concat on D)
  - row-parallel output projection: per-core Wp rows [128, C]; host sums the
    8 partial [B*T, C] outputs (the all-reduce) and adds bp.

Step-1 scheduling rewrite over the 156us baseline:
  - x DMAs split per batch (b0 chunked for fast lead-in, b1-3 one 3D DMA
    each) so batch 1+ data lands before the PE needs it (the old bulk load
    stalled the PE 4us at b1 and re-throttled HAM to half clock for 17us)
  - v is computed transposed directly (x chunk stationary, Wv moving) ->
    no PE transposes, no psvt pool; the freed PSUM banks give the scores
    [128, 2, 512] tiles (both heads per tile)
  - exp merged: one ACTIVATE per (s, piece) covering both heads (48 instead
    of 96 instrs; each carries a 352-cycle pipeline-fill overhead)
  - denominators: DMA straight from PSUM ones-row to packed [16,2,32], DVE
    reciprocal, DRAM bounce, [64,512] broadcasts (no ACT den copies)
  - lag-1 proj pipeline (pop prev batch's halves during each batch) and
    per-tile 256KB out DMAs dispatched from GpSimd -> short tail drain
  - engine split: ACT=exp only, DVE=all psum copies+normalize muls+recip,
    GPSIMD=masks+out-DMA dispatch, Sync=in-DMAs+den bounce
"""

import ml_dtypes
import numpy as np

B, T, C = 4, 1024, 1024
H, D = 16, 64
NCORES = 8
HPC = H // NCORES      # heads per core = 2
D2 = HPC * D           # 128
BT = B * T
SCALE = 1.0 / np.sqrt(np.float32(C))  # 1/32
BF16 = ml_dtypes.bfloat16

_compiled = None

NWARM = 12


def _split_multi_waits(nc, mybir, maxw=1):
    """Walrus in this container encodes at most one sync wait per
    instruction (fp32 self-loading matmuls and drains overflow).  Hoist
    excess waits onto same-engine NoOps inserted just before."""
    for fn in nc.m.functions:
        for bb in fn.blocks:
            new = []
            for inst in bb.instructions:
                si = inst.sync_info
                waits = list(si.on_wait) if (si is not None and si.on_wait) else []
                if len(waits) > maxw:
                    extra, keep = waits[:-maxw], waits[-maxw:]
                    for j, w in enumerate(extra):
                        new.append(
                            mybir.InstNoOp(
                                name=f"{inst.name}-wsplit{j}",
                                engine=inst.engine,
                                sync_info=mybir.SyncInfo(on_wait=[w], on_update=[]),
                                bass_nofuse=True,
                            )
                        )
                    inst.sync_info = mybir.SyncInfo(
                        on_wait=keep,
                        on_update=list(si.on_update) if si.on_update else [],
                    )
                new.append(inst)
            bb.instructions = new


def _build():
    import concourse.bass as bass
    import concourse.mybir as mybir
    import concourse.tile as tile

    f32 = mybir.dt.float32
    bf = mybir.dt.bfloat16
    EXP = mybir.ActivationFunctionType.Exp

    nc = bass.Bass("TRN2", target_bir_lowering=False, debug=False, num_devices=NCORES)

    xT_d = nc.dram_tensor("xT", [C, BT], bf, kind="ExternalInput").ap()
    # host pre-shuffles each weight to [p, k, m] so the DMA is contiguous
    wq_d = nc.dram_tensor("wq", [128, C // 128, D2], bf, kind="ExternalInput").ap()
    wk_d = nc.dram_tensor("wk", [128, C // 128, D2], bf, kind="ExternalInput").ap()
    wv_d = nc.dram_tensor("wv", [128, C // 128, D2], bf, kind="ExternalInput").ap()
    wp_d = nc.dram_tensor("wp", [D2, C], bf, kind="ExternalInput").ap()
    mask_d = nc.dram_tensor("mask", [128, HPC, 128], bf, kind="ExternalInput").ap()
    ident_d = nc.dram_tensor("ident", [128, 128], f32, kind="ExternalInput").ap()
    out_d = nc.dram_tensor("out", [BT, C], bf, kind="ExternalOutput").ap()

    KC = C // 128  # 8 contraction chunks over C
    NS = T // 128  # 8 s-chunks

    import concourse.bass as _bass

    with tile.TileContext(nc) as tc:
        with (
            tc.tile_pool(name="const", bufs=1) as constp,
            tc.tile_pool(name="xin", bufs=1) as xinp,
            tc.tile_pool(name="qkv", bufs=2) as qkvp,
            tc.tile_pool(name="vaug", bufs=2) as vaugp,
            tc.tile_pool(name="exps", bufs=10) as expp,
            tc.tile_pool(name="smalls", bufs=2) as smallp,
            tc.tile_pool(name="outt", bufs=3) as outtp,
            tc.tile_pool(name="pout", bufs=3) as poutp,
            tc.tile_pool(name="dram", bufs=2, space="DRAM") as dramp,
            tc.tile_pool(name="psc", bufs=2, space="PSUM") as pscp,
            tc.tile_pool(name="psatt", bufs=2, space="PSUM") as psattp,
            tc.tile_pool(name="psproj", bufs=1, space="PSUM") as psprojp,
        ):
            # ---- constants / warmup ----
            wq_s = constp.tile([128, KC, D2], bf, tag="wq")
            wk_s = constp.tile([128, KC, D2], bf, tag="wk")
            wv_s = constp.tile([128, KC, D2], bf, tag="wv")
            wp_s = constp.tile([128, C], bf, tag="wp")
            mask_s = constp.tile([128, HPC, 128], bf, tag="mask")
            ident = constp.tile([128, 128], f32, tag="ident")
            junk = constp.tile([128, 512], bf, tag="junk")

            # Warm-up: junk matmuls with no DMA deps fill the PE stream while
            # inputs land, so HAM un-throttles before real work.  memset on
            # GpSimd so it issues during the framework preamble.
            nc.gpsimd.memset(junk[:], 0.0)
            for i in range(NWARM):
                pw = pscp.tile([128, 2, 512], f32, tag="sc", name=f"warm{i}")
                nc.tensor.matmul(
                    pw[:, 0, :], junk[:, 0:128], junk[:], start=True, stop=True
                )

            # Input DMAs, critical-first: wq/wk + batch-0 x chunks gate the
            # first real matmuls; batches 1-3 land as one 3D DMA each.
            xba = xinp.tile([128, KC, BT], bf, tag="xba", name="xba", bufs=1)
            nc.sync.dma_start(wq_s[:], wq_d)
            nc.sync.dma_start(wk_s[:], wk_d)
            for k in range(KC):
                nc.sync.dma_start(
                    xba[:, k, 0:T], xT_d[k * 128:(k + 1) * 128, 0:T]
                )
                if k == 1:
                    nc.sync.dma_start(wv_s[:], wv_d)
                if k == 2:
                    nc.sync.dma_start(mask_s[:], mask_d)
                if k == 3:
                    nc.sync.dma_start(wp_s[:], wp_d)
                if k == 4:
                    nc.sync.dma_start(ident[:], ident_d)
            for b in range(1, B):
                nc.sync.dma_start(
                    xba[:, :, b * T:(b + 1) * T],
                    xT_d[:, b * T:(b + 1) * T].rearrange(
                        "(k p) t -> p k t", p=128
                    ),
                )

            def emit_qk(b, xb):
                """q and k for batch b, interleaved per k-chunk so the lead-in
                tracks the b0 chunk DMAs.  One [128,2,512] psum per half:
                q in [:,0,:], k in [:,1,:]; one merged copy per half."""
                qkT = qkvp.tile([128, 2, T], bf, tag="qkT", name=f"qkT{b}")
                for half in range(2):
                    ps = pscp.tile([128, 2, 512], f32, tag="sc",
                                   name=f"qk{b}_{half}")
                    cs = slice(half * 512, (half + 1) * 512)
                    for k in range(KC):
                        nc.tensor.matmul(
                            ps[:, 0, :], wq_s[:, k, :], xb[:, k, cs],
                            start=(k == 0), stop=(k == KC - 1),
                        )
                        nc.tensor.matmul(
                            ps[:, 1, :], wk_s[:, k, :], xb[:, k, cs],
                            start=(k == 0), stop=(k == KC - 1),
                        )
                    nc.vector.tensor_copy(qkT[:, :, cs], ps[:])
                return qkT[:, 0, :], qkT[:, 1, :]

            def emit_v_mms(b, xb):
                """v = Wv.T @ x, both 512-col halves into one sc tile (Wv
                stationary: only 8 LDWEIGHTS), one merged f32 copy to SBUF."""
                pv = pscp.tile([128, 2, 512], f32, tag="sc", name=f"v{b}")
                for half in range(2):
                    cs = slice(half * 512, (half + 1) * 512)
                    for k in range(KC):
                        nc.tensor.matmul(
                            pv[:, half, :], wv_s[:, k, :], xb[:, k, cs],
                            start=(k == 0), stop=(k == KC - 1),
                        )
                vT = qkvp.tile([128, T], f32, tag="vT", name=f"vT{b}")
                nc.vector.tensor_copy(vT[:], pv[:])
                return vT

            def emit_vtrans(b, vT, vaug, quad):
                """PE-transpose 4 t-chunks (f32, 2cyc/row) into one att-pool
                bank, then one merged DVE copy into vaug."""
                pv = psattp.tile([128, 512], f32, tag="att",
                                 name=f"vt{b}_{quad}")
                for i in range(4):
                    s = quad * 4 + i
                    nc.tensor.transpose(
                        pv[:, i * 128:(i + 1) * 128],
                        vT[:, s * 128:(s + 1) * 128], ident[:]
                    )
                nc.vector.tensor_copy(
                    vaug[:, quad * 4:quad * 4 + 4, :, 0:64],
                    pv[:].rearrange("p (s h d) -> p s h d", s=4, h=HPC),
                )

            def emit_scores_s(b, s, qT, kT, exs):
                """Scores for chunk s, both heads.  ex layout: col j of
                ex[:,h,:] is t = s0 + j.  One exp ACTIVATE per piece covers
                both heads; diagonal-block mask is one GPSIMD op."""
                s0 = s * 128
                d1 = max(0, s0 - 512)
                ex = expp.tile([128, HPC, 1024], bf, tag="ex", bufs=10,
                               name=f"ex{b}_{s}")
                exs[s] = ex
                if s < 4:  # t-half0 piece: cols [s0, 512)
                    w0 = 512 - s0
                    pa = pscp.tile([128, 2, 512], f32, tag="sc",
                                   name=f"sc{b}_{s}a")
                    for h in range(HPC):
                        hp = slice(h * 64, (h + 1) * 64)
                        nc.tensor.matmul(
                            pa[:, h, 0:w0], kT[hp, s0:s0 + 128],
                            qT[hp, s0:512], start=True, stop=True,
                        )
                    nc.scalar.activation(
                        ex[:, :, 0:w0], pa[:, :, 0:w0], EXP, scale=float(SCALE)
                    )
                # t-half1 piece: cols [max(512, s0), 1024)
                w1 = 512 - d1
                pb = pscp.tile([128, 2, 512], f32, tag="sc",
                               name=f"sc{b}_{s}b")
                for h in range(HPC):
                    hp = slice(h * 64, (h + 1) * 64)
                    nc.tensor.matmul(
                        pb[:, h, 0:w1], kT[hp, s0:s0 + 128],
                        qT[hp, 512 + d1:T], start=True, stop=True,
                    )
                nc.scalar.activation(
                    ex[:, :, 512 - s0 + d1:T - s0], pb[:, :, 0:w1],
                    EXP, scale=float(SCALE),
                )
                nc.gpsimd.tensor_mul(ex[:, :, 0:128], ex[:, :, 0:128], mask_s[:])

            def emit_po0_s(b, s, vaug, exs, po0):
                assert 0 <= s <= 3
                s0 = s * 128
                for h in range(HPC):
                    nc.tensor.matmul(
                        po0[h][0:65, s0:512],
                        vaug[:, s, h, 0:65],
                        exs[s][:, h, 0:512 - s0],
                        start=(s == 0),
                        stop=(s == 3),
                    )

            def emit_den(b, half, den_srcs, scr_rec):
                """DMA the psum ones-rows to packed [16,2,32], DVE recip,
                bounce through DRAM for contiguous broadcast source."""
                den2 = smallp.tile([1, HPC, 512], f32, tag="den2", bufs=2,
                                   name=f"dn{b}_{half}")
                nc.scalar.copy(den2[:, 0, :], den_srcs[0])
                nc.scalar.copy(den2[:, 1, :], den_srcs[1])
                packed = smallp.tile([16, HPC, 32], f32, tag="packed",
                                     name=f"pk{b}_{half}")
                nc.sync.dma_start(packed[:], den2[:])
                recp = smallp.tile([16, HPC, 32], f32, tag="recp",
                                   name=f"rc{b}_{half}")
                nc.vector.reciprocal(recp[:], packed[:])
                nc.sync.dma_start(
                    scr_rec[0, :].rearrange("(p h f) -> p h f", p=16, h=HPC),
                    recp[:],
                )

            def emit_norm_half(b, half, po_h, outT2, scr_rec):
                """Broadcast each head's reciprocals to [64,512] and apply."""
                t0 = half * 512
                for h in range(HPC):
                    hp = slice(h * 64, (h + 1) * 64)
                    rec2 = smallp.tile([64, 512], f32, tag="rec2", bufs=4,
                                       name=f"rec2_{b}_{half}_{h}")
                    nc.sync.dma_start(
                        rec2[:],
                        _bass.AP(
                            scr_rec[:].tensor,
                            scr_rec[:].offset + 512 * h,
                            [[0, 64], [1, 512]],
                        ),
                    )
                    nc.vector.tensor_mul(
                        outT2[hp, t0:t0 + 512], po_h[h][0:64, 0:512], rec2[:]
                    )

            def emit_proj_tile(pb, o2, i, tt, ob, flush=None):
                """One [128,2,512] psum per proj tile: both 512-col halves of
                Wp accumulate side by side, then ONE merged [128,1024] cast.
                flush='act'/'alt' uses the (idle) scores-pool banks and routes
                casts to ACT / alternating engines for the end-of-kernel
                drain; default uses the single-buffer proj bank with DVE
                casts (1-in-4 on ACT)."""
                if flush is None:
                    pp = psprojp.tile([128, 2, 512], f32, tag="proj",
                                      name=f"pj{pb}_{tt}")
                    eng = "v"
                else:
                    pp = pscp.tile([128, 2, 512], f32, tag="sc",
                                   name=f"pjf{pb}_{tt}")
                    eng = "s" if (flush == "act" or i % 2 == 1) else "v"
                for ct in range(2):
                    nc.tensor.matmul(
                        pp[:, ct, :],
                        o2[:, tt * 128:(tt + 1) * 128],
                        wp_s[:, ct * 512:(ct + 1) * 512],
                        start=True, stop=True,
                    )
                if eng == "s":
                    nc.scalar.copy(ob[:, i, :], pp[:])
                else:
                    nc.vector.tensor_copy(ob[:, i, :], pp[:])
                # per-tile 256KB out DMA, dispatched from GpSimd
                r0 = pb * T + (tt // 4) * 512 + i * 128
                nc.gpsimd.dma_start(out_d[r0:r0 + 128, :], ob[:, i, :])

            def proj_half_ob(pb, half):
                return poutp.tile([128, 4, C], bf, tag="ob",
                                  name=f"ob{pb}_{half}")

            # ---- main pipeline ----
            pend = []  # pending proj halves: (batch, outT2, half)

            def pop_proj(b, lag=1, flush=None):
                if pend and pend[0][0] <= b - lag:
                    pb, o2, half = pend.pop(0)
                    ob = proj_half_ob(pb, half)

                    def mk(i, tt):
                        def thunk():
                            emit_proj_tile(pb, o2, i, tt, ob, flush=flush)
                        return thunk

                    return [
                        mk(i, tt)
                        for i, tt in enumerate(range(half * 4, half * 4 + 4))
                    ]
                return []

            state = {}  # per-batch carry for finish_batch

            def finish_batch(b):
                """po1 + half1 normalize chain for batch b.  Called after the
                NEXT batch's qk matmuls so po1 (which waits on b's last exp)
                has PE work in front of it, and the sc ring has drained."""
                vaug, outT2, exs, scr1 = state.pop(b)
                last = b == B - 1
                po1t = pscp.tile([128, 2, 512], f32, tag="sc", name=f"po1_{b}")
                po1 = [po1t[:, h, :] for h in range(HPC)]
                slot_b = pop_proj(b, flush="act" if last else None)
                for h in range(HPC):
                    for s in range(NS):
                        s0 = s * 128
                        d1 = max(0, s0 - 512)
                        nc.tensor.matmul(
                            po1[h][0:65, d1:512],
                            vaug[:, s, h, 0:65],
                            exs[s][:, h, 512 - s0 + d1:T - s0],
                            start=(s == 0),
                            stop=(s == NS - 1),
                        )
                    if h == 0:
                        for t in slot_b[:2]:
                            t()
                emit_den(b, 1, [po1[h][64:65, 0:512] for h in range(HPC)], scr1)
                # Evacuate po1 to SBUF right away: the sc-ring reuse otherwise
                # blocks the next batch's psums behind this batch's bounce.
                po1s = smallp.tile([65, HPC, 512], bf, tag="po1s", bufs=2,
                                   name=f"po1s{b}")
                nc.vector.tensor_copy(po1s[:], po1t[0:65, :, :])
                for t in slot_b[2:]:
                    t()
                if last:
                    # flush own half0 (casts on ACT so DVE reaches the half1
                    # normalize muls the moment the bounce lands)
                    for t in pop_proj(b, lag=0, flush="act"):
                        t()
                emit_norm_half(b, 1, [po1s[:, h, :] for h in range(HPC)],
                               outT2, scr1)
                pend.append((b, outT2, 1))

            for b in range(B):
                xb = xba[:, :, b * T:(b + 1) * T]
                vaug = vaugp.tile([128, NS, HPC, 66], bf, tag="vaug",
                                  name=f"vaug{b}")
                nc.vector.memset(vaug[:, :, :, 64:65], 1.0)
                outT2 = outtp.tile([128, T], bf, tag="outT2", name=f"outT2_{b}")
                exs = {}
                scr0 = dramp.tile([1, 1024], f32, tag="scr", name=f"scr{b}_0")
                scr1 = dramp.tile([1, 1024], f32, tag="scr2", name=f"scr{b}_1")
                state[b] = (vaug, outT2, exs, scr1)

                qT, kT = emit_qk(b, xb)
                if b > 0:
                    finish_batch(b - 1)
                # scores s=0 early: ACT exp head start over the v PE work.
                emit_scores_s(b, 0, qT, kT, exs)
                vT = emit_v_mms(b, xb)
                emit_scores_s(b, 1, qT, kT, exs)
                emit_vtrans(b, vT, vaug, 0)
                emit_vtrans(b, vT, vaug, 1)

                slot_a = pop_proj(b)  # prev batch half1, spread into s-loop
                po0 = [
                    psattp.tile([128, 512], f32, tag="att", name=f"po0_{b}_{h}")
                    for h in range(HPC)
                ]
                for s in range(2, NS):
                    emit_scores_s(b, s, qT, kT, exs)
                    if s - 2 <= 3:
                        emit_po0_s(b, s - 2, vaug, exs, po0)
                    if s - 2 == 3:
                        # po0 complete -> launch half0 denominator chain
                        emit_den(b, 0, [po0[h][64:65, 0:512] for h in range(HPC)],
                                 scr0)
                    if s == 6:
                        emit_norm_half(b, 0, po0, outT2, scr0)
                        pend.append((b, outT2, 0))
                    if 0 <= s - 3 < len(slot_a):
                        slot_a[s - 3]()  # PE backfill, no exp dep

            finish_batch(B - 1)
            while pend:
                pb, o2, half = pend.pop(0)
                ob = proj_half_ob(pb, half)
                for i, tt in enumerate(range(half * 4, half * 4 + 4)):
                    emit_proj_tile(pb, o2, i, tt, ob, flush="alt")

    _split_multi_waits(nc, mybir)
    return nc


def _get_compiled():
    global _compiled
    if _compiled is None:
        _compiled = _build()
    return _compiled


def _shuf_w(W, h0):
    # [H, C, D] head-pair slice -> [C, D2] -> pre-shuffled [p, k, m] so the
    # device DMA is one contiguous [128, 1024] transfer per weight.
    w = np.asarray(W[h0:h0 + HPC], dtype=np.float32).transpose(1, 0, 2).reshape(C, D2)
    return np.ascontiguousarray(
        w.reshape(C // 128, 128, D2).transpose(1, 0, 2)
    ).astype(BF16)


def _make_in_maps(x, Wq, Wk, Wv, Wp):
    xT = np.ascontiguousarray(
        np.asarray(x, dtype=np.float32).reshape(BT, C).T
    ).astype(BF16)  # [C, BT]
    mask1 = np.triu(np.ones((128, 128), dtype=BF16))  # keep j >= i
    mask = np.ascontiguousarray(
        np.stack([mask1] * HPC, axis=1)
    )  # [128, HPC, 128]
    identf = np.eye(128, dtype=np.float32)
    in_maps = []
    for i in range(NCORES):
        h0 = i * HPC
        wp = np.ascontiguousarray(
            np.asarray(Wp, dtype=np.float32)[h0 * D:(h0 + HPC) * D, :]
        ).astype(BF16)
        in_maps.append(
            {"xT": xT, "wq": _shuf_w(Wq, h0), "wk": _shuf_w(Wk, h0),
             "wv": _shuf_w(Wv, h0), "wp": wp, "mask": mask, "ident": identf}
        )
    return in_maps


def run(x, Wq, Wk, Wv, Wp, bp, trace=False, trace_cores=None):
    """Returns (full_output [B,T,C], BassKernelResults)."""
    from concourse.bass_utils import run_bass_kernel_spmd

    nc = _get_compiled()
    in_maps = _make_in_maps(x, Wq, Wk, Wv, Wp)
    kw = {}
    if trace:
        kw = {"trace": True, "trace_cores": trace_cores or [0]}
    res = run_bass_kernel_spmd(nc, in_maps, list(range(NCORES)), **kw)
    acc = np.zeros((BT, C), dtype=np.float32)
    for i in range(NCORES):
        acc += np.asarray(res.results[i]["out"], dtype=np.float32)
    acc += np.asarray(bp, dtype=np.float32)[None, :]
    return acc.reshape(B, T, C), res


def kernel(x, Wq, Wk, Wv, Wp, bp):
    out, _ = run(x, Wq, Wk, Wv, Wp, bp)
    return out


# revision 22
# speedup vs baseline: 1.0541x; 1.0541x over previous
"""Multi-head causal attention (B=4, T=1024, C=1024, H=16, D=64) on 8 TRN2 cores.

Sharding: tensor-parallel over heads. Core i owns heads {2i, 2i+1}:
  - x is replicated (sent pre-transposed as xT [C, B*T], bf16)
  - Wq/Wk/Wv sharded over heads -> per-core [C, 128] (2 heads concat on D)
  - row-parallel output projection: per-core Wp rows [128, C]; host sums the
    8 partial [B*T, C] outputs (the all-reduce) and adds bp.

Step-1 scheduling rewrite over the 156us baseline:
  - x DMAs split per batch (b0 chunked for fast lead-in, b1-3 one 3D DMA
    each) so batch 1+ data lands before the PE needs it (the old bulk load
    stalled the PE 4us at b1 and re-throttled HAM to half clock for 17us)
  - v is computed transposed directly (x chunk stationary, Wv moving) ->
    no PE transposes, no psvt pool; the freed PSUM banks give the scores
    [128, 2, 512] tiles (both heads per tile)
  - exp merged: one ACTIVATE per (s, piece) covering both heads (48 instead
    of 96 instrs; each carries a 352-cycle pipeline-fill overhead)
  - denominators: DMA straight from PSUM ones-row to packed [16,2,32], DVE
    reciprocal, DRAM bounce, [64,512] broadcasts (no ACT den copies)
  - lag-1 proj pipeline (pop prev batch's halves during each batch) and
    per-tile 256KB out DMAs dispatched from GpSimd -> short tail drain
  - engine split: ACT=exp only, DVE=all psum copies+normalize muls+recip,
    GPSIMD=masks+out-DMA dispatch, Sync=in-DMAs+den bounce
"""

import ml_dtypes
import numpy as np

B, T, C = 4, 1024, 1024
H, D = 16, 64
NCORES = 8
HPC = H // NCORES      # heads per core = 2
D2 = HPC * D           # 128
BT = B * T
SCALE = 1.0 / np.sqrt(np.float32(C))  # 1/32
BF16 = ml_dtypes.bfloat16

_compiled = None

NWARM = 10


def _split_multi_waits(nc, mybir, maxw=1):
    """Walrus in this container encodes at most one sync wait per
    instruction (fp32 self-loading matmuls and drains overflow).  Hoist
    excess waits onto same-engine NoOps inserted just before."""
    for fn in nc.m.functions:
        for bb in fn.blocks:
            new = []
            for inst in bb.instructions:
                si = inst.sync_info
                waits = list(si.on_wait) if (si is not None and si.on_wait) else []
                if len(waits) > maxw:
                    extra, keep = waits[:-maxw], waits[-maxw:]
                    for j, w in enumerate(extra):
                        new.append(
                            mybir.InstNoOp(
                                name=f"{inst.name}-wsplit{j}",
                                engine=inst.engine,
                                sync_info=mybir.SyncInfo(on_wait=[w], on_update=[]),
                                bass_nofuse=True,
                            )
                        )
                    inst.sync_info = mybir.SyncInfo(
                        on_wait=keep,
                        on_update=list(si.on_update) if si.on_update else [],
                    )
                new.append(inst)
            bb.instructions = new


def _build():
    import concourse.bass as bass
    import concourse.mybir as mybir
    import concourse.tile as tile

    f32 = mybir.dt.float32
    bf = mybir.dt.bfloat16
    EXP = mybir.ActivationFunctionType.Exp

    nc = bass.Bass("TRN2", target_bir_lowering=False, debug=False, num_devices=NCORES)

    xT_d = nc.dram_tensor("xT", [C, BT], bf, kind="ExternalInput").ap()
    # host pre-shuffles each weight to [p, k, m] so the DMA is contiguous
    wq_d = nc.dram_tensor("wq", [128, C // 128, D2], bf, kind="ExternalInput").ap()
    wk_d = nc.dram_tensor("wk", [128, C // 128, D2], bf, kind="ExternalInput").ap()
    wv_d = nc.dram_tensor("wv", [128, C // 128, D2], bf, kind="ExternalInput").ap()
    wp_d = nc.dram_tensor("wp", [D2, C], bf, kind="ExternalInput").ap()
    mask_d = nc.dram_tensor("mask", [128, HPC, 128], bf, kind="ExternalInput").ap()
    ident_d = nc.dram_tensor("ident", [128, 128], f32, kind="ExternalInput").ap()
    out_d = nc.dram_tensor("out", [BT, C], bf, kind="ExternalOutput").ap()

    KC = C // 128  # 8 contraction chunks over C
    NS = T // 128  # 8 s-chunks

    import concourse.bass as _bass

    with tile.TileContext(nc) as tc:
        with (
            tc.tile_pool(name="const", bufs=1) as constp,
            tc.tile_pool(name="xin", bufs=1) as xinp,
            tc.tile_pool(name="qkv", bufs=2) as qkvp,
            tc.tile_pool(name="vaug", bufs=2) as vaugp,
            tc.tile_pool(name="exps", bufs=10) as expp,
            tc.tile_pool(name="smalls", bufs=2) as smallp,
            tc.tile_pool(name="outt", bufs=3) as outtp,
            tc.tile_pool(name="pout", bufs=3) as poutp,
            tc.tile_pool(name="dram", bufs=2, space="DRAM") as dramp,
            tc.tile_pool(name="psc", bufs=2, space="PSUM") as pscp,
            tc.tile_pool(name="psatt", bufs=2, space="PSUM") as psattp,
            tc.tile_pool(name="psproj", bufs=1, space="PSUM") as psprojp,
        ):
            # ---- constants / warmup ----
            wq_s = constp.tile([128, KC, D2], bf, tag="wq")
            wk_s = constp.tile([128, KC, D2], bf, tag="wk")
            wv_s = constp.tile([128, KC, D2], bf, tag="wv")
            wp_s = constp.tile([128, C], bf, tag="wp")
            mask_s = constp.tile([128, HPC, 128], bf, tag="mask")
            ident = constp.tile([128, 128], f32, tag="ident")
            junk = constp.tile([128, 512], bf, tag="junk")

            # Warm-up: junk matmuls with no DMA deps fill the PE stream while
            # inputs land, so HAM un-throttles before real work.  memset on
            # GpSimd so it issues during the framework preamble.
            nc.gpsimd.memset(junk[:], 0.0)
            for i in range(NWARM):
                pw = pscp.tile([128, 2, 512], f32, tag="sc", name=f"warm{i}")
                nc.tensor.matmul(
                    pw[:, 0, :], junk[:, 0:128], junk[:], start=True, stop=True
                )

            # Input DMAs, critical-first: wq/wk + batch-0 x chunks gate the
            # first real matmuls; batches 1-3 land as one 3D DMA each.
            xba = xinp.tile([128, KC, BT], bf, tag="xba", name="xba", bufs=1)
            nc.sync.dma_start(wq_s[:], wq_d)
            nc.sync.dma_start(wk_s[:], wk_d)
            for k in range(KC):
                nc.sync.dma_start(
                    xba[:, k, 0:T], xT_d[k * 128:(k + 1) * 128, 0:T]
                )
                if k == 1:
                    nc.sync.dma_start(wv_s[:], wv_d)
                if k == 2:
                    nc.sync.dma_start(mask_s[:], mask_d)
                if k == 3:
                    nc.sync.dma_start(wp_s[:], wp_d)
                if k == 4:
                    nc.sync.dma_start(ident[:], ident_d)
            for b in range(1, B):
                nc.sync.dma_start(
                    xba[:, :, b * T:(b + 1) * T],
                    xT_d[:, b * T:(b + 1) * T].rearrange(
                        "(k p) t -> p k t", p=128
                    ),
                )

            def emit_qk(b, xb):
                """q and k for batch b, interleaved per k-chunk so the lead-in
                tracks the b0 chunk DMAs.  One [128,2,512] psum per half:
                q in [:,0,:], k in [:,1,:]; one merged copy per half."""
                qkT = qkvp.tile([128, 2, T], bf, tag="qkT", name=f"qkT{b}")
                for half in range(2):
                    ps = pscp.tile([128, 2, 512], f32, tag="sc",
                                   name=f"qk{b}_{half}")
                    cs = slice(half * 512, (half + 1) * 512)
                    for k in range(KC):
                        nc.tensor.matmul(
                            ps[:, 0, :], wq_s[:, k, :], xb[:, k, cs],
                            start=(k == 0), stop=(k == KC - 1),
                        )
                        nc.tensor.matmul(
                            ps[:, 1, :], wk_s[:, k, :], xb[:, k, cs],
                            start=(k == 0), stop=(k == KC - 1),
                        )
                    nc.vector.tensor_copy(qkT[:, :, cs], ps[:])
                return qkT[:, 0, :], qkT[:, 1, :]

            def emit_v_mms(b, xb):
                """v = Wv.T @ x, both 512-col halves into one sc tile (Wv
                stationary: only 8 LDWEIGHTS), one merged f32 copy to SBUF."""
                pv = pscp.tile([128, 2, 512], f32, tag="sc", name=f"v{b}")
                for half in range(2):
                    cs = slice(half * 512, (half + 1) * 512)
                    for k in range(KC):
                        nc.tensor.matmul(
                            pv[:, half, :], wv_s[:, k, :], xb[:, k, cs],
                            start=(k == 0), stop=(k == KC - 1),
                        )
                vT = qkvp.tile([128, T], f32, tag="vT", name=f"vT{b}")
                nc.vector.tensor_copy(vT[:], pv[:])
                return vT

            def emit_vtrans(b, vT, vaug, quad):
                """PE-transpose 4 t-chunks (f32, 2cyc/row) into one att-pool
                bank, then one merged DVE copy into vaug."""
                pv = psattp.tile([128, 512], f32, tag="att",
                                 name=f"vt{b}_{quad}")
                for i in range(4):
                    s = quad * 4 + i
                    nc.tensor.transpose(
                        pv[:, i * 128:(i + 1) * 128],
                        vT[:, s * 128:(s + 1) * 128], ident[:]
                    )
                nc.vector.tensor_copy(
                    vaug[:, quad * 4:quad * 4 + 4, :, 0:64],
                    pv[:].rearrange("p (s h d) -> p s h d", s=4, h=HPC),
                )

            def emit_scores_s(b, s, qT, kT, exs):
                """Scores for chunk s, both heads.  ex layout: col j of
                ex[:,h,:] is t = s0 + j.  One exp ACTIVATE per piece covers
                both heads; diagonal-block mask is one GPSIMD op."""
                s0 = s * 128
                d1 = max(0, s0 - 512)
                ex = expp.tile([128, HPC, 1024], bf, tag="ex", bufs=10,
                               name=f"ex{b}_{s}")
                exs[s] = ex
                if s < 4:  # t-half0 piece: cols [s0, 512)
                    w0 = 512 - s0
                    pa = pscp.tile([128, 2, 512], f32, tag="sc",
                                   name=f"sc{b}_{s}a")
                    for h in range(HPC):
                        hp = slice(h * 64, (h + 1) * 64)
                        nc.tensor.matmul(
                            pa[:, h, 0:w0], kT[hp, s0:s0 + 128],
                            qT[hp, s0:512], start=True, stop=True,
                        )
                    nc.scalar.activation(
                        ex[:, :, 0:w0], pa[:, :, 0:w0], EXP, scale=float(SCALE)
                    )
                # t-half1 piece: cols [max(512, s0), 1024)
                w1 = 512 - d1
                pb = pscp.tile([128, 2, 512], f32, tag="sc",
                               name=f"sc{b}_{s}b")
                for h in range(HPC):
                    hp = slice(h * 64, (h + 1) * 64)
                    nc.tensor.matmul(
                        pb[:, h, 0:w1], kT[hp, s0:s0 + 128],
                        qT[hp, 512 + d1:T], start=True, stop=True,
                    )
                nc.scalar.activation(
                    ex[:, :, 512 - s0 + d1:T - s0], pb[:, :, 0:w1],
                    EXP, scale=float(SCALE),
                )
                nc.gpsimd.tensor_mul(ex[:, :, 0:128], ex[:, :, 0:128], mask_s[:])

            def emit_po0_s(b, s, vaug, exs, po0):
                assert 0 <= s <= 3
                s0 = s * 128
                for h in range(HPC):
                    nc.tensor.matmul(
                        po0[h][0:65, s0:512],
                        vaug[:, s, h, 0:65],
                        exs[s][:, h, 0:512 - s0],
                        start=(s == 0),
                        stop=(s == 3),
                    )

            def emit_den(b, half, den_srcs, scr_rec):
                """DMA the psum ones-rows to packed [16,2,32], DVE recip,
                bounce through DRAM for contiguous broadcast source."""
                den2 = smallp.tile([1, HPC, 512], f32, tag="den2", bufs=2,
                                   name=f"dn{b}_{half}")
                nc.vector.tensor_copy(den2[:, 0, :], den_srcs[0])
                nc.vector.tensor_copy(den2[:, 1, :], den_srcs[1])
                packed = smallp.tile([16, HPC, 32], f32, tag="packed",
                                     name=f"pk{b}_{half}")
                nc.sync.dma_start(packed[:], den2[:])
                recp = smallp.tile([16, HPC, 32], f32, tag="recp",
                                   name=f"rc{b}_{half}")
                nc.vector.reciprocal(recp[:], packed[:])
                nc.sync.dma_start(
                    scr_rec[0, :].rearrange("(p h f) -> p h f", p=16, h=HPC),
                    recp[:],
                )

            def emit_norm_half(b, half, po_h, outT2, scr_rec):
                """Broadcast each head's reciprocals to [64,512] and apply."""
                t0 = half * 512
                for h in range(HPC):
                    hp = slice(h * 64, (h + 1) * 64)
                    rec2 = smallp.tile([64, 512], f32, tag="rec2", bufs=4,
                                       name=f"rec2_{b}_{half}_{h}")
                    nc.sync.dma_start(
                        rec2[:],
                        _bass.AP(
                            scr_rec[:].tensor,
                            scr_rec[:].offset + 512 * h,
                            [[0, 64], [1, 512]],
                        ),
                    )
                    nc.vector.tensor_mul(
                        outT2[hp, t0:t0 + 512], po_h[h][0:64, 0:512], rec2[:]
                    )

            def emit_proj_tile(pb, o2, i, tt, ob, flush=None):
                """One [128,2,512] psum per proj tile: both 512-col halves of
                Wp accumulate side by side, then ONE merged [128,1024] cast.
                flush='act'/'alt' uses the (idle) scores-pool banks and routes
                casts to ACT / alternating engines for the end-of-kernel
                drain; default uses the single-buffer proj bank with DVE
                casts (1-in-4 on ACT)."""
                if flush is None:
                    pp = psprojp.tile([128, 2, 512], f32, tag="proj",
                                      name=f"pj{pb}_{tt}")
                    eng = "v"
                else:
                    pp = pscp.tile([128, 2, 512], f32, tag="sc",
                                   name=f"pjf{pb}_{tt}")
                    eng = "s" if (flush == "act" or i % 2 == 1) else "v"
                for ct in range(2):
                    nc.tensor.matmul(
                        pp[:, ct, :],
                        o2[:, tt * 128:(tt + 1) * 128],
                        wp_s[:, ct * 512:(ct + 1) * 512],
                        start=True, stop=True,
                    )
                if eng == "s":
                    nc.scalar.copy(ob[:, i, :], pp[:])
                else:
                    nc.vector.tensor_copy(ob[:, i, :], pp[:])
                # per-tile 256KB out DMA, dispatched from GpSimd
                r0 = pb * T + (tt // 4) * 512 + i * 128
                nc.gpsimd.dma_start(out_d[r0:r0 + 128, :], ob[:, i, :])

            def proj_half_ob(pb, half):
                return poutp.tile([128, 4, C], bf, tag="ob",
                                  name=f"ob{pb}_{half}")

            # ---- main pipeline ----
            pend = []  # pending proj halves: (batch, outT2, half)

            def pop_proj(b, lag=1, flush=None):
                if pend and pend[0][0] <= b - lag:
                    pb, o2, half = pend.pop(0)
                    ob = proj_half_ob(pb, half)

                    def mk(i, tt):
                        def thunk():
                            emit_proj_tile(pb, o2, i, tt, ob, flush=flush)
                        return thunk

                    return [
                        mk(i, tt)
                        for i, tt in enumerate(range(half * 4, half * 4 + 4))
                    ]
                return []

            state = {}  # per-batch carry for finish_batch

            def finish_batch(b):
                """po1 + half1 normalize chain for batch b.  Called after the
                NEXT batch's qk matmuls so po1 (which waits on b's last exp)
                has PE work in front of it, and the sc ring has drained."""
                vaug, outT2, exs, scr1 = state.pop(b)
                last = b == B - 1
                po1t = pscp.tile([128, 2, 512], f32, tag="sc", name=f"po1_{b}")
                po1 = [po1t[:, h, :] for h in range(HPC)]
                slot_b = pop_proj(b, flush="act" if last else None)
                for h in range(HPC):
                    for s in range(NS):
                        s0 = s * 128
                        d1 = max(0, s0 - 512)
                        nc.tensor.matmul(
                            po1[h][0:65, d1:512],
                            vaug[:, s, h, 0:65],
                            exs[s][:, h, 512 - s0 + d1:T - s0],
                            start=(s == 0),
                            stop=(s == NS - 1),
                        )
                    if h == 0:
                        for t in slot_b[:2]:
                            t()
                emit_den(b, 1, [po1[h][64:65, 0:512] for h in range(HPC)], scr1)
                # Evacuate po1 to SBUF right away: the sc-ring reuse otherwise
                # blocks the next batch's psums behind this batch's bounce.
                po1s = smallp.tile([65, HPC, 512], bf, tag="po1s", bufs=2,
                                   name=f"po1s{b}")
                nc.vector.tensor_copy(po1s[:], po1t[0:65, :, :])
                for t in slot_b[2:]:
                    t()
                if last:
                    # flush own half0 (casts on ACT so DVE reaches the half1
                    # normalize muls the moment the bounce lands)
                    for t in pop_proj(b, lag=0, flush="act"):
                        t()
                emit_norm_half(b, 1, [po1s[:, h, :] for h in range(HPC)],
                               outT2, scr1)
                pend.append((b, outT2, 1))

            for b in range(B):
                xb = xba[:, :, b * T:(b + 1) * T]
                vaug = vaugp.tile([128, NS, HPC, 66], bf, tag="vaug",
                                  name=f"vaug{b}")
                nc.vector.memset(vaug[:, :, :, 64:65], 1.0)
                outT2 = outtp.tile([128, T], bf, tag="outT2", name=f"outT2_{b}")
                exs = {}
                scr0 = dramp.tile([1, 1024], f32, tag="scr", name=f"scr{b}_0")
                scr1 = dramp.tile([1, 1024], f32, tag="scr2", name=f"scr{b}_1")
                state[b] = (vaug, outT2, exs, scr1)

                qT, kT = emit_qk(b, xb)
                if b > 0:
                    finish_batch(b - 1)
                # scores s=0 early: ACT exp head start over the v PE work.
                emit_scores_s(b, 0, qT, kT, exs)
                vT = emit_v_mms(b, xb)
                emit_scores_s(b, 1, qT, kT, exs)
                emit_vtrans(b, vT, vaug, 0)
                emit_vtrans(b, vT, vaug, 1)

                slot_a = pop_proj(b)  # prev batch half1, spread into s-loop
                po0 = [
                    psattp.tile([128, 512], f32, tag="att", name=f"po0_{b}_{h}")
                    for h in range(HPC)
                ]
                for s in range(2, NS):
                    emit_scores_s(b, s, qT, kT, exs)
                    if s - 2 <= 3:
                        emit_po0_s(b, s - 2, vaug, exs, po0)
                    if s - 2 == 3:
                        # po0 complete -> launch half0 denominator chain
                        emit_den(b, 0, [po0[h][64:65, 0:512] for h in range(HPC)],
                                 scr0)
                    if s == 6:
                        emit_norm_half(b, 0, po0, outT2, scr0)
                        pend.append((b, outT2, 0))
                    if 0 <= s - 3 < len(slot_a):
                        slot_a[s - 3]()  # PE backfill, no exp dep

            finish_batch(B - 1)
            while pend:
                pb, o2, half = pend.pop(0)
                ob = proj_half_ob(pb, half)
                for i, tt in enumerate(range(half * 4, half * 4 + 4)):
                    emit_proj_tile(pb, o2, i, tt, ob, flush="alt")

    _split_multi_waits(nc, mybir)
    return nc


def _get_compiled():
    global _compiled
    if _compiled is None:
        _compiled = _build()
    return _compiled


def _shuf_w(W, h0):
    # [H, C, D] head-pair slice -> [C, D2] -> pre-shuffled [p, k, m] so the
    # device DMA is one contiguous [128, 1024] transfer per weight.
    w = np.asarray(W[h0:h0 + HPC], dtype=np.float32).transpose(1, 0, 2).reshape(C, D2)
    return np.ascontiguousarray(
        w.reshape(C // 128, 128, D2).transpose(1, 0, 2)
    ).astype(BF16)


def _make_in_maps(x, Wq, Wk, Wv, Wp):
    xT = np.ascontiguousarray(
        np.asarray(x, dtype=np.float32).reshape(BT, C).T
    ).astype(BF16)  # [C, BT]
    mask1 = np.triu(np.ones((128, 128), dtype=BF16))  # keep j >= i
    mask = np.ascontiguousarray(
        np.stack([mask1] * HPC, axis=1)
    )  # [128, HPC, 128]
    identf = np.eye(128, dtype=np.float32)
    in_maps = []
    for i in range(NCORES):
        h0 = i * HPC
        wp = np.ascontiguousarray(
            np.asarray(Wp, dtype=np.float32)[h0 * D:(h0 + HPC) * D, :]
        ).astype(BF16)
        in_maps.append(
            {"xT": xT, "wq": _shuf_w(Wq, h0), "wk": _shuf_w(Wk, h0),
             "wv": _shuf_w(Wv, h0), "wp": wp, "mask": mask, "ident": identf}
        )
    return in_maps


def run(x, Wq, Wk, Wv, Wp, bp, trace=False, trace_cores=None):
    """Returns (full_output [B,T,C], BassKernelResults)."""
    from concourse.bass_utils import run_bass_kernel_spmd

    nc = _get_compiled()
    in_maps = _make_in_maps(x, Wq, Wk, Wv, Wp)
    kw = {}
    if trace:
        kw = {"trace": True, "trace_cores": trace_cores or [0]}
    res = run_bass_kernel_spmd(nc, in_maps, list(range(NCORES)), **kw)
    acc = np.zeros((BT, C), dtype=np.float32)
    for i in range(NCORES):
        acc += np.asarray(res.results[i]["out"], dtype=np.float32)
    acc += np.asarray(bp, dtype=np.float32)[None, :]
    return acc.reshape(B, T, C), res


def kernel(x, Wq, Wk, Wv, Wp, bp):
    out, _ = run(x, Wq, Wk, Wv, Wp, bp)
    return out


# revision 31
# speedup vs baseline: 1.0588x; 1.0044x over previous
"""Multi-head causal attention (B=4, T=1024, C=1024, H=16, D=64) on 8 TRN2 cores.

Sharding: tensor-parallel over heads. Core i owns heads {2i, 2i+1}:
  - x is replicated (sent pre-transposed as xT [C, B*T], bf16)
  - Wq/Wk/Wv sharded over heads -> per-core [C, 128] (2 heads concat on D)
  - row-parallel output projection: per-core Wp rows [128, C]; host sums the
    8 partial [B*T, C] outputs (the all-reduce) and adds bp.

Step-1 scheduling rewrite over the 156us baseline:
  - x DMAs split per batch (b0 chunked for fast lead-in, b1-3 one 3D DMA
    each) so batch 1+ data lands before the PE needs it (the old bulk load
    stalled the PE 4us at b1 and re-throttled HAM to half clock for 17us)
  - v is computed transposed directly (x chunk stationary, Wv moving) ->
    no PE transposes, no psvt pool; the freed PSUM banks give the scores
    [128, 2, 512] tiles (both heads per tile)
  - exp merged: one ACTIVATE per (s, piece) covering both heads (48 instead
    of 96 instrs; each carries a 352-cycle pipeline-fill overhead)
  - denominators: DMA straight from PSUM ones-row to packed [16,2,32], DVE
    reciprocal, DRAM bounce, [64,512] broadcasts (no ACT den copies)
  - lag-1 proj pipeline (pop prev batch's halves during each batch) and
    per-tile 256KB out DMAs dispatched from GpSimd -> short tail drain
  - engine split: ACT=exp only, DVE=all psum copies+normalize muls+recip,
    GPSIMD=masks+out-DMA dispatch, Sync=in-DMAs+den bounce
"""

import ml_dtypes
import numpy as np

B, T, C = 4, 1024, 1024
H, D = 16, 64
NCORES = 8
HPC = H // NCORES      # heads per core = 2
D2 = HPC * D           # 128
BT = B * T
SCALE = 1.0 / np.sqrt(np.float32(C))  # 1/32
W8SCALE = 32.0  # wq/wk prescale before fp8 cast (dodges fp8 subnormals)
# q,k both carry W8SCALE -> scores carry W8SCALE^2; fold into the exp scale
EXP_SCALE = float(SCALE) / (W8SCALE * W8SCALE)
BF16 = ml_dtypes.bfloat16

_compiled = None

NWARM = 10


def _split_multi_waits(nc, mybir, maxw=1):
    """Walrus in this container encodes at most one sync wait per
    instruction (fp32 self-loading matmuls and drains overflow).  Hoist
    excess waits onto same-engine NoOps inserted just before."""
    for fn in nc.m.functions:
        for bb in fn.blocks:
            new = []
            for inst in bb.instructions:
                si = inst.sync_info
                waits = list(si.on_wait) if (si is not None and si.on_wait) else []
                if len(waits) > maxw:
                    extra, keep = waits[:-maxw], waits[-maxw:]
                    for j, w in enumerate(extra):
                        new.append(
                            mybir.InstNoOp(
                                name=f"{inst.name}-wsplit{j}",
                                engine=inst.engine,
                                sync_info=mybir.SyncInfo(on_wait=[w], on_update=[]),
                                bass_nofuse=True,
                            )
                        )
                    inst.sync_info = mybir.SyncInfo(
                        on_wait=keep,
                        on_update=list(si.on_update) if si.on_update else [],
                    )
                new.append(inst)
            bb.instructions = new


def _build():
    import concourse.bass as bass
    import concourse.mybir as mybir
    import concourse.tile as tile

    f32 = mybir.dt.float32
    bf = mybir.dt.bfloat16
    EXP = mybir.ActivationFunctionType.Exp

    nc = bass.Bass("TRN2", target_bir_lowering=False, debug=False, num_devices=NCORES)

    f8 = mybir.dt.float8e4
    DR = mybir.MatmulPerfMode.DoubleRow

    xT_d = nc.dram_tensor("xT", [C, BT], bf, kind="ExternalInput").ap()
    xf8_d = nc.dram_tensor("xf8", [C, BT], f8, kind="ExternalInput").ap()
    # host pre-shuffles each weight to [p, k, m] so the DMA is contiguous;
    # wq/wk are fp8 (scaled x32 to dodge fp8 subnormals; exp scale absorbs it)
    wq_d = nc.dram_tensor("wq", [128, C // 128, D2], f8, kind="ExternalInput").ap()
    wk_d = nc.dram_tensor("wk", [128, C // 128, D2], f8, kind="ExternalInput").ap()
    wv_d = nc.dram_tensor("wv", [128, C // 128, D2], bf, kind="ExternalInput").ap()
    wp_d = nc.dram_tensor("wp", [D2, C], bf, kind="ExternalInput").ap()
    mask_d = nc.dram_tensor("mask", [128, HPC, 128], bf, kind="ExternalInput").ap()
    ident_d = nc.dram_tensor("ident", [128, 128], f32, kind="ExternalInput").ap()
    out_d = nc.dram_tensor("out", [BT, C], bf, kind="ExternalOutput").ap()

    KC = C // 128  # 8 contraction chunks over C
    NS = T // 128  # 8 s-chunks

    import concourse.bass as _bass

    with tile.TileContext(nc) as tc:
        with (
            tc.tile_pool(name="const", bufs=1) as constp,
            tc.tile_pool(name="xin", bufs=1) as xinp,
            tc.tile_pool(name="qkv", bufs=2) as qkvp,
            tc.tile_pool(name="vaug", bufs=2) as vaugp,
            tc.tile_pool(name="exps", bufs=9) as expp,
            tc.tile_pool(name="smalls", bufs=2) as smallp,
            tc.tile_pool(name="outt", bufs=3) as outtp,
            tc.tile_pool(name="pout", bufs=2) as poutp,
            tc.tile_pool(name="dram", bufs=2, space="DRAM") as dramp,
            tc.tile_pool(name="psc", bufs=2, space="PSUM") as pscp,
            tc.tile_pool(name="psatt", bufs=2, space="PSUM") as psattp,
            tc.tile_pool(name="psproj", bufs=1, space="PSUM") as psprojp,
        ):
            # ---- constants / warmup ----
            wq_s = constp.tile([128, KC, D2], f8, tag="wq")
            wk_s = constp.tile([128, KC, D2], f8, tag="wk")
            wv_s = constp.tile([128, KC, D2], bf, tag="wv")
            wp_s = constp.tile([128, C], bf, tag="wp")
            mask_s = constp.tile([128, HPC, 128], bf, tag="mask")
            ident = constp.tile([128, 128], f32, tag="ident")
            junk = constp.tile([128, 512], bf, tag="junk")

            # Warm-up: junk matmuls with no DMA deps fill the PE stream while
            # inputs land, so HAM un-throttles before real work.  memset on
            # GpSimd so it issues during the framework preamble.
            nc.gpsimd.memset(junk[:], 0.0)
            for i in range(NWARM):
                pw = pscp.tile([128, 2, 512], f32, tag="sc", name=f"warm{i}")
                nc.tensor.matmul(
                    pw[:, 0, :], junk[:, 0:128], junk[:], start=True, stop=True
                )

            # Input DMAs, critical-first: wq/wk + batch-0 fp8 x chunks gate
            # the first qk matmuls, then b0 bf16 x (for v); batches 1-3 land
            # as one 3D DMA each per dtype.
            xba = xinp.tile([128, KC, BT], bf, tag="xba", name="xba", bufs=1)
            xf8 = xinp.tile([128, KC, BT], f8, tag="xf8", name="xf8", bufs=1)
            nc.sync.dma_start(wq_s[:], wq_d)
            nc.sync.dma_start(wk_s[:], wk_d)
            for k in range(KC):
                nc.sync.dma_start(
                    xf8[:, k, 0:T], xf8_d[k * 128:(k + 1) * 128, 0:T]
                )
                if k == 1:
                    nc.sync.dma_start(wv_s[:], wv_d)
            for k in range(KC):
                nc.sync.dma_start(
                    xba[:, k, 0:T], xT_d[k * 128:(k + 1) * 128, 0:T]
                )
                if k == 1:
                    nc.sync.dma_start(mask_s[:], mask_d)
                if k == 2:
                    nc.sync.dma_start(wp_s[:], wp_d)
                if k == 3:
                    nc.sync.dma_start(ident[:], ident_d)
            for b in range(1, B):
                nc.sync.dma_start(
                    xf8[:, :, b * T:(b + 1) * T],
                    xf8_d[:, b * T:(b + 1) * T].rearrange(
                        "(k p) t -> p k t", p=128
                    ),
                )
                nc.sync.dma_start(
                    xba[:, :, b * T:(b + 1) * T],
                    xT_d[:, b * T:(b + 1) * T].rearrange(
                        "(k p) t -> p k t", p=128
                    ),
                )

            def emit_qk(b, xq):
                """q and k for batch b via fp8 DoubleRow (256-deep contraction
                per pass, 2x column rate), q/k interleaved per chunk-pair.
                One [128,2,512] psum per half; one merged copy per half."""
                qkT = qkvp.tile([128, 2, T], bf, tag="qkT", name=f"qkT{b}")
                for half in range(2):
                    ps = pscp.tile([128, 2, 512], f32, tag="sc",
                                   name=f"qk{b}_{half}")
                    cs = slice(half * 512, (half + 1) * 512)
                    for j in range(KC // 2):
                        kp = slice(2 * j, 2 * j + 2)
                        nc.tensor.matmul(
                            ps[:, 0, :], wq_s[:, kp, :], xq[:, kp, cs],
                            start=(j == 0), stop=(j == KC // 2 - 1),
                            perf_mode=DR,
                        )
                        nc.tensor.matmul(
                            ps[:, 1, :], wk_s[:, kp, :], xq[:, kp, cs],
                            start=(j == 0), stop=(j == KC // 2 - 1),
                            perf_mode=DR,
                        )
                    nc.vector.tensor_copy(qkT[:, :, cs], ps[:])
                return qkT[:, 0, :], qkT[:, 1, :]

            def emit_v_mms(b, xb):
                """v = Wv.T @ x, both 512-col halves into one sc tile (Wv
                stationary: only 8 LDWEIGHTS), one merged f32 copy to SBUF."""
                pv = pscp.tile([128, 2, 512], f32, tag="sc", name=f"v{b}")
                for half in range(2):
                    cs = slice(half * 512, (half + 1) * 512)
                    for k in range(KC):
                        nc.tensor.matmul(
                            pv[:, half, :], wv_s[:, k, :], xb[:, k, cs],
                            start=(k == 0), stop=(k == KC - 1),
                        )
                vT = qkvp.tile([128, T], f32, tag="vT", name=f"vT{b}")
                nc.vector.tensor_copy(vT[:], pv[:])
                return vT

            def emit_vtrans(b, vT, vaug, quad):
                """PE-transpose 4 t-chunks (f32, 2cyc/row) into one att-pool
                bank, then one merged DVE copy into vaug."""
                pv = psattp.tile([128, 512], f32, tag="att",
                                 name=f"vt{b}_{quad}")
                for i in range(4):
                    s = quad * 4 + i
                    nc.tensor.transpose(
                        pv[:, i * 128:(i + 1) * 128],
                        vT[:, s * 128:(s + 1) * 128], ident[:]
                    )
                nc.vector.tensor_copy(
                    vaug[:, quad * 4:quad * 4 + 4, :, 0:64],
                    pv[:].rearrange("p (s h d) -> p s h d", s=4, h=HPC),
                )

            def emit_scores_s(b, s, qT, kT, exs):
                """Scores for chunk s, both heads.  ex layout: col j of
                ex[:,h,:] is t = s0 + j.  One exp ACTIVATE per piece covers
                both heads; diagonal-block mask is one GPSIMD op."""
                s0 = s * 128
                d1 = max(0, s0 - 512)
                ex = expp.tile([128, HPC, 1024], bf, tag="ex", bufs=9,
                               name=f"ex{b}_{s}")
                exs[s] = ex
                if s < 4:  # t-half0 piece: cols [s0, 512)
                    w0 = 512 - s0
                    pa = pscp.tile([128, 2, 512], f32, tag="sc",
                                   name=f"sc{b}_{s}a")
                    for h in range(HPC):
                        hp = slice(h * 64, (h + 1) * 64)
                        nc.tensor.matmul(
                            pa[:, h, 0:w0], kT[hp, s0:s0 + 128],
                            qT[hp, s0:512], start=True, stop=True,
                        )
                    nc.scalar.activation(
                        ex[:, :, 0:w0], pa[:, :, 0:w0], EXP, scale=EXP_SCALE
                    )
                # t-half1 piece: cols [max(512, s0), 1024)
                w1 = 512 - d1
                pb = pscp.tile([128, 2, 512], f32, tag="sc",
                               name=f"sc{b}_{s}b")
                for h in range(HPC):
                    hp = slice(h * 64, (h + 1) * 64)
                    nc.tensor.matmul(
                        pb[:, h, 0:w1], kT[hp, s0:s0 + 128],
                        qT[hp, 512 + d1:T], start=True, stop=True,
                    )
                nc.scalar.activation(
                    ex[:, :, 512 - s0 + d1:T - s0], pb[:, :, 0:w1],
                    EXP, scale=EXP_SCALE,
                )
                nc.gpsimd.tensor_mul(ex[:, :, 0:128], ex[:, :, 0:128], mask_s[:])

            def emit_po0_s(b, s, vaug, exs, po0):
                assert 0 <= s <= 3
                s0 = s * 128
                for h in range(HPC):
                    nc.tensor.matmul(
                        po0[h][0:65, s0:512],
                        vaug[:, s, h, 0:65],
                        exs[s][:, h, 0:512 - s0],
                        start=(s == 0),
                        stop=(s == 3),
                    )

            def emit_den(b, half, den_srcs, scr_rec):
                """DMA the psum ones-rows to packed [16,2,32], DVE recip,
                bounce through DRAM for contiguous broadcast source."""
                den2 = smallp.tile([1, HPC, 512], f32, tag="den2", bufs=2,
                                   name=f"dn{b}_{half}")
                nc.vector.tensor_copy(den2[:, 0, :], den_srcs[0])
                nc.vector.tensor_copy(den2[:, 1, :], den_srcs[1])
                packed = smallp.tile([16, HPC, 32], f32, tag="packed",
                                     name=f"pk{b}_{half}")
                nc.sync.dma_start(packed[:], den2[:])
                recp = smallp.tile([16, HPC, 32], f32, tag="recp",
                                   name=f"rc{b}_{half}")
                nc.vector.reciprocal(recp[:], packed[:])
                nc.sync.dma_start(
                    scr_rec[0, :].rearrange("(p h f) -> p h f", p=16, h=HPC),
                    recp[:],
                )

            def emit_norm_half(b, half, po_h, outT2, scr_rec):
                """Broadcast each head's reciprocals to [64,512] and apply."""
                t0 = half * 512
                for h in range(HPC):
                    hp = slice(h * 64, (h + 1) * 64)
                    rec2 = smallp.tile([64, 512], f32, tag="rec2", bufs=4,
                                       name=f"rec2_{b}_{half}_{h}")
                    nc.sync.dma_start(
                        rec2[:],
                        _bass.AP(
                            scr_rec[:].tensor,
                            scr_rec[:].offset + 512 * h,
                            [[0, 64], [1, 512]],
                        ),
                    )
                    nc.vector.tensor_mul(
                        outT2[hp, t0:t0 + 512], po_h[h][0:64, 0:512], rec2[:]
                    )

            def emit_proj_tile(pb, o2, i, tt, ob, flush=None):
                """One [128,2,512] psum per proj tile: both 512-col halves of
                Wp accumulate side by side, then ONE merged [128,1024] cast.
                flush='act'/'alt' uses the (idle) scores-pool banks and routes
                casts to ACT / alternating engines for the end-of-kernel
                drain; default uses the single-buffer proj bank with DVE
                casts (1-in-4 on ACT)."""
                if flush is None:
                    pp = psprojp.tile([128, 2, 512], f32, tag="proj",
                                      name=f"pj{pb}_{tt}")
                    eng = "v"
                else:
                    pp = pscp.tile([128, 2, 512], f32, tag="sc",
                                   name=f"pjf{pb}_{tt}")
                    eng = "s" if (flush == "act" or i % 2 == 1) else "v"
                for ct in range(2):
                    nc.tensor.matmul(
                        pp[:, ct, :],
                        o2[:, tt * 128:(tt + 1) * 128],
                        wp_s[:, ct * 512:(ct + 1) * 512],
                        start=True, stop=True,
                    )
                if eng == "s":
                    nc.scalar.copy(ob[:, i, :], pp[:])
                else:
                    nc.vector.tensor_copy(ob[:, i, :], pp[:])
                # per-tile 256KB out DMA, dispatched from GpSimd
                r0 = pb * T + (tt // 4) * 512 + i * 128
                nc.gpsimd.dma_start(out_d[r0:r0 + 128, :], ob[:, i, :])

            def proj_half_ob(pb, half):
                return poutp.tile([128, 4, C], bf, tag="ob",
                                  name=f"ob{pb}_{half}")

            # ---- main pipeline ----
            pend = []  # pending proj halves: (batch, outT2, half)

            def pop_proj(b, lag=1, flush=None):
                if pend and pend[0][0] <= b - lag:
                    pb, o2, half = pend.pop(0)
                    ob = proj_half_ob(pb, half)

                    def mk(i, tt):
                        def thunk():
                            emit_proj_tile(pb, o2, i, tt, ob, flush=flush)
                        return thunk

                    return [
                        mk(i, tt)
                        for i, tt in enumerate(range(half * 4, half * 4 + 4))
                    ]
                return []

            state = {}  # per-batch carry for finish_batch

            def finish_batch(b):
                """po1 + half1 normalize chain for batch b.  Called after the
                NEXT batch's qk matmuls so po1 (which waits on b's last exp)
                has PE work in front of it, and the sc ring has drained."""
                vaug, outT2, exs, scr1 = state.pop(b)
                last = b == B - 1
                po1t = pscp.tile([128, 2, 512], f32, tag="sc", name=f"po1_{b}")
                po1 = [po1t[:, h, :] for h in range(HPC)]
                slot_b = pop_proj(b, flush="act" if last else None)
                for h in range(HPC):
                    for s in range(NS):
                        s0 = s * 128
                        d1 = max(0, s0 - 512)
                        nc.tensor.matmul(
                            po1[h][0:65, d1:512],
                            vaug[:, s, h, 0:65],
                            exs[s][:, h, 512 - s0 + d1:T - s0],
                            start=(s == 0),
                            stop=(s == NS - 1),
                        )
                    if h == 0:
                        for t in slot_b[:2]:
                            t()
                emit_den(b, 1, [po1[h][64:65, 0:512] for h in range(HPC)], scr1)
                # Evacuate po1 to SBUF right away: the sc-ring reuse otherwise
                # blocks the next batch's psums behind this batch's bounce.
                po1s = smallp.tile([65, HPC, 512], bf, tag="po1s", bufs=2,
                                   name=f"po1s{b}")
                nc.vector.tensor_copy(po1s[:], po1t[0:65, :, :])
                for t in slot_b[2:]:
                    t()
                if last:
                    # flush own half0 (casts on ACT so DVE reaches the half1
                    # normalize muls the moment the bounce lands)
                    for t in pop_proj(b, lag=0, flush="act"):
                        t()
                emit_norm_half(b, 1, [po1s[:, h, :] for h in range(HPC)],
                               outT2, scr1)
                pend.append((b, outT2, 1))

            for b in range(B):
                xb = xba[:, :, b * T:(b + 1) * T]
                vaug = vaugp.tile([128, NS, HPC, 66], bf, tag="vaug",
                                  name=f"vaug{b}")
                nc.vector.memset(vaug[:, :, :, 64:65], 1.0)
                outT2 = outtp.tile([128, T], bf, tag="outT2", name=f"outT2_{b}")
                exs = {}
                scr0 = dramp.tile([1, 1024], f32, tag="scr", name=f"scr{b}_0")
                scr1 = dramp.tile([1, 1024], f32, tag="scr2", name=f"scr{b}_1")
                state[b] = (vaug, outT2, exs, scr1)

                qT, kT = emit_qk(b, xf8[:, :, b * T:(b + 1) * T])
                if b > 0:
                    finish_batch(b - 1)
                # scores s=0 early: ACT exp head start over the v PE work.
                emit_scores_s(b, 0, qT, kT, exs)
                vT = emit_v_mms(b, xb)
                emit_scores_s(b, 1, qT, kT, exs)
                emit_vtrans(b, vT, vaug, 0)
                emit_vtrans(b, vT, vaug, 1)

                slot_a = pop_proj(b)  # prev batch half1, spread into s-loop
                po0 = [
                    psattp.tile([128, 512], f32, tag="att", name=f"po0_{b}_{h}")
                    for h in range(HPC)
                ]
                for s in range(2, NS):
                    emit_scores_s(b, s, qT, kT, exs)
                    if s - 2 <= 3:
                        emit_po0_s(b, s - 2, vaug, exs, po0)
                    if s - 2 == 3:
                        # po0 complete -> launch half0 denominator chain
                        emit_den(b, 0, [po0[h][64:65, 0:512] for h in range(HPC)],
                                 scr0)
                    if s == 6:
                        emit_norm_half(b, 0, po0, outT2, scr0)
                        pend.append((b, outT2, 0))
                    if 0 <= s - 3 < len(slot_a):
                        slot_a[s - 3]()  # PE backfill, no exp dep

            finish_batch(B - 1)
            while pend:
                pb, o2, half = pend.pop(0)
                ob = proj_half_ob(pb, half)
                for i, tt in enumerate(range(half * 4, half * 4 + 4)):
                    emit_proj_tile(pb, o2, i, tt, ob, flush="alt")

    _split_multi_waits(nc, mybir)
    return nc


def _get_compiled():
    global _compiled
    if _compiled is None:
        _compiled = _build()
    return _compiled


def _shuf_w(W, h0, dtype=BF16, scale=1.0):
    # [H, C, D] head-pair slice -> [C, D2] -> pre-shuffled [p, k, m] so the
    # device DMA is one contiguous [128, 1024] transfer per weight.
    w = np.asarray(W[h0:h0 + HPC], dtype=np.float32).transpose(1, 0, 2).reshape(C, D2)
    return (np.ascontiguousarray(
        w.reshape(C // 128, 128, D2).transpose(1, 0, 2)
    ) * scale).astype(dtype)


def _make_in_maps(x, Wq, Wk, Wv, Wp):
    F8 = ml_dtypes.float8_e4m3
    xTf = np.ascontiguousarray(
        np.asarray(x, dtype=np.float32).reshape(BT, C).T
    )  # [C, BT]
    xT = xTf.astype(BF16)
    xf8 = xTf.astype(F8)
    mask1 = np.triu(np.ones((128, 128), dtype=BF16))  # keep j >= i
    mask = np.ascontiguousarray(
        np.stack([mask1] * HPC, axis=1)
    )  # [128, HPC, 128]
    identf = np.eye(128, dtype=np.float32)
    in_maps = []
    for i in range(NCORES):
        h0 = i * HPC
        wp = np.ascontiguousarray(
            np.asarray(Wp, dtype=np.float32)[h0 * D:(h0 + HPC) * D, :]
        ).astype(BF16)
        in_maps.append(
            {"xT": xT, "xf8": xf8,
             "wq": _shuf_w(Wq, h0, F8, W8SCALE),
             "wk": _shuf_w(Wk, h0, F8, W8SCALE),
             "wv": _shuf_w(Wv, h0), "wp": wp, "mask": mask, "ident": identf}
        )
    return in_maps


def run(x, Wq, Wk, Wv, Wp, bp, trace=False, trace_cores=None):
    """Returns (full_output [B,T,C], BassKernelResults)."""
    from concourse.bass_utils import run_bass_kernel_spmd

    nc = _get_compiled()
    in_maps = _make_in_maps(x, Wq, Wk, Wv, Wp)
    kw = {}
    if trace:
        kw = {"trace": True, "trace_cores": trace_cores or [0]}
    res = run_bass_kernel_spmd(nc, in_maps, list(range(NCORES)), **kw)
    acc = np.zeros((BT, C), dtype=np.float32)
    for i in range(NCORES):
        acc += np.asarray(res.results[i]["out"], dtype=np.float32)
    acc += np.asarray(bp, dtype=np.float32)[None, :]
    return acc.reshape(B, T, C), res


def kernel(x, Wq, Wk, Wv, Wp, bp):
    out, _ = run(x, Wq, Wk, Wv, Wp, bp)
    return out
